# revision 1
# baseline (speedup 1.0000x reference)
# Trainium2 Bass kernel for nn_ConvAttention (B=2, N=4096 (64x64), C=128, H=4, DH=32).
#
# Sharding: 8 cores = (batch b in {0,1}) x (query-row-block rb in {0..3}).
# Each core computes the full k/v separable convs for its batch image, the q conv
# for its 1024-row block (+halo rows, host-supplied), then flash-style attention
# over its query rows for all 4 heads, and the output projection for its block.
# Host gathers the 8 blocks into the full [2, 4096, 128] output.
#
# Conv fold (host): depthwise 3x3 + bias + BN + pointwise 1x1 collapse into
# 9 accumulated matmuls per branch: W_t[c,f] = dwk_t[c]*A[c]*pwk[c,f] with
# A = g*rsqrt(var+eps); all bias terms fold into a per-branch const (q/k, added
# on PSUM->SBUF evacuation) or into the final projection bias (v).
#
# Attention: scores computed transposed (S^T[j,i] = kT_h.T @ qT_h, K=DH=32,
# 4 heads packed into the PE array via tile_position row groups) so that the
# softmax'd probabilities P^T[j,i] are directly the rhs of the AV matmul
# (out_catT[c,i] = v_h.T @ P^T_h, col-tiled 4 heads into one PSUM tile) and the
# denominator matmul (ones.T @ P^T_h -> denominator replicated across the 32
# partitions of each head). exp runs on ACT with scale=DH^-0.5 folded into the
# activation affine; no max-subtraction (validated: |scores*scale| << 80).

import numpy as np

import concourse.bass as bass
import concourse.tile as tile
from concourse import bacc, mybir
from concourse import bass_utils

B, L, W, C, H = 2, 64, 64, 128, 4
DH = C // H          # 32
N = L * W            # 4096
NB = N // 4          # 1024 query rows per core
SCALE = float(DH) ** -0.5
BN_EPS = 1e-3
P = 128
IC = 256             # i-chunk (query columns per inner pass)
N_IC = NB // IC      # 4
NJT = N // P         # 32 j tiles

F32 = mybir.dt.float32
F32R = mybir.dt.float32r
BF16 = mybir.dt.bfloat16


def _r(ap):
    return ap.bitcast(F32R)


def build_program():
    """Build and compile the SPMD single-core program (same on all 8 cores)."""
    nc = bacc.Bacc(
        "TRN2",
        target_bir_lowering=False,
        debug=False,
        enable_asserts=False,
        num_devices=8,
    )

    def din(name, shape, dt=F32):
        return nc.dram_tensor(name, list(shape), dt, kind="ExternalInput").ap()

    d = {
        'xpad': din("xpad", (P, 66 * 66), F32R),
        'xpad_q': din("xpad_q", (P, 18 * 66), F32R),
        'wq': din("wq", (P, 9, P), F32R),
        'wk': din("wk", (P, 9, P), F32R),
        'wvdiag': din("wvdiag", (P, 9, P), F32R),
        'pwkv': din("pwkv", (P, P)),
        'cq': din("cq", (P, 1)),
        'ck': din("ck", (P, 1)),
        'out_w': din("out_w", (P, P)),
        'ob_rep': din("ob_rep", (P, P)),
    }
    d_out = nc.dram_tensor("out", [NB, C], F32, kind="ExternalOutput").ap()

    with tile.TileContext(nc) as tc:
        _build(tc, d, d_out)

    nc.compile()
    return nc


def _build(tc, d, d_out):
    from contextlib import ExitStack
    nc = tc.nc
    Exp = mybir.ActivationFunctionType.Exp

    ctx = ExitStack()
    persist = ctx.enter_context(tc.tile_pool(name="persist", bufs=1))
    cpsum = ctx.enter_context(tc.tile_pool(name="cpsum", bufs=2, space="PSUM"))
    scp = ctx.enter_context(tc.tile_pool(name="scp", bufs=2, space="PSUM"))
    avp = ctx.enter_context(tc.tile_pool(name="avp", bufs=1, space="PSUM"))
    denp = ctx.enter_context(tc.tile_pool(name="denp", bufs=1, space="PSUM"))
    ptp = ctx.enter_context(tc.tile_pool(name="ptp", bufs=3))
    nrm = ctx.enter_context(tc.tile_pool(name="nrm", bufs=2))

    # ---- load inputs ----
    xpad = persist.tile([P, 66, 66], F32R)
    nc.sync.dma_start(out=xpad, in_=d['xpad'].rearrange("p (a b) -> p a b", b=66))
    xq = persist.tile([P, 18, 66], F32R)
    nc.sync.dma_start(out=xq, in_=d['xpad_q'].rearrange("p (a b) -> p a b", b=66))
    wq = persist.tile([P, 9, P], F32R)
    nc.sync.dma_start(out=wq, in_=d['wq'])
    wk = persist.tile([P, 9, P], F32R)
    nc.sync.dma_start(out=wk, in_=d['wk'])
    wvdiag = persist.tile([P, 9, P], F32R)
    nc.sync.dma_start(out=wvdiag, in_=d['wvdiag'])
    pwkv = persist.tile([P, P], F32)
    nc.sync.dma_start(out=pwkv, in_=d['pwkv'])
    cq = persist.tile([P, 1], F32)
    nc.sync.dma_start(out=cq, in_=d['cq'])
    ck = persist.tile([P, 1], F32)
    nc.sync.dma_start(out=ck, in_=d['ck'])
    ow = persist.tile([P, P], F32)
    nc.sync.dma_start(out=ow, in_=d['out_w'])
    ob = persist.tile([P, P], F32)
    nc.sync.dma_start(out=ob, in_=d['ob_rep'])

    ones = persist.tile([P, DH], BF16)
    nc.vector.memset(ones, 1.0)

    kT = persist.tile([P, N], BF16)        # [f, n] all heads stacked
    qblk = persist.tile([P, H, NB], BF16)  # per-head zero-padded q
    nc.vector.memset(qblk, 0.0)
    yvT = persist.tile([P, N], F32)       # v depthwise output [c, n]
    vsb = persist.tile([P, NJT, P], BF16)  # vsb[p, t, f] = v[t*128+p, f]
    ostage = persist.tile([P, NB // P, P], F32)

    # ---- q/k fused conv: out_T[f, n] accumulated over 9 taps ----
    # chunk s covers image rows [8s, 8s+8); tap t=(dl,dw) reads padded rows
    # 8s+dl .. 8s+dl+8, padded cols dw .. dw+64.
    def qk_conv(wgt, cst, outT, src, nchunks, per_head=False):
        for s in range(nchunks):
            ps = cpsum.tile([P, 512], F32, tag="cps", name="cps")
            for t in range(9):
                dl, dw = t // 3, t % 3
                win = src[:, 8 * s + dl:8 * s + dl + 8, dw:dw + 64]
                nc.tensor.matmul(ps, wgt[:, t], win,
                                 start=(t == 0), stop=(t == 8))
            if per_head:
                for h in range(H):
                    nc.vector.tensor_scalar_add(
                        outT[DH * h:DH * (h + 1), h, 512 * s:512 * (s + 1)],
                        ps[DH * h:DH * (h + 1), :], cst[DH * h:DH * (h + 1)])
            else:
                nc.vector.tensor_scalar_add(outT[:, 512 * s:512 * (s + 1)], ps, cst)

    qk_conv(wk, ck, kT, xpad, 8)
    qk_conv(wq, cq, qblk, xq, 2, per_head=True)

    # ---- v depthwise via diagonal tap matrices, then pointwise ----
    for s in range(8):
        ps = cpsum.tile([P, 512], F32, tag="cps")
        for t in range(9):
            dl, dw = t // 3, t % 3
            win = xpad[:, 8 * s + dl:8 * s + dl + 8, dw:dw + 64]
            nc.tensor.matmul(ps, wvdiag[:, t], win,
                             start=(t == 0), stop=(t == 8))
        nc.vector.tensor_copy(yvT[:, 512 * s:512 * (s + 1)], ps)
    for s in range(NJT):
        pv = cpsum.tile([P, 512], F32, tag="cps", name="pv")[:, :P]
        nc.tensor.matmul(pv, yvT[:, P * s:P * (s + 1)], pwkv,
                         start=True, stop=True)
        nc.vector.tensor_copy(vsb[:, s, :], pv)

    # ---- attention ----
    for ic in range(N_IC):
        av = avp.tile([P, IC], F32, tag="av")
        den = denp.tile([P, IC], F32, tag="den")
        for jt in range(NJT):
            sc = scp.tile([P, H, IC], F32, tag="sc")
            for hp in range(H // 2):
                nc.tensor.matmul(
                    sc[:, 2 * hp:2 * hp + 2, :],
                    kT[:, P * jt:P * (jt + 1)],
                    qblk[:, 2 * hp:2 * hp + 2, IC * ic:IC * (ic + 1)],
                    start=True, stop=True,
                )
            pt = ptp.tile([P, H, IC], BF16, tag="pt")
            nc.scalar.activation(out=pt, in_=sc, func=Exp, scale=SCALE)
            for h in range(H):
                nc.tensor.matmul(
                    av[DH * h:DH * (h + 1), :],
                    vsb[:, jt, DH * h:DH * (h + 1)],
                    pt[:, h],
                    start=(jt == 0), stop=(jt == NJT - 1),
                    tile_position=(0, DH * h),
                    skip_group_check=True,
                )
            for h in range(H):
                nc.tensor.matmul(
                    den[DH * h:DH * (h + 1), :],
                    ones,
                    pt[:, h],
                    start=(jt == 0), stop=(jt == NJT - 1),
                    tile_position=(0, DH * h),
                    skip_group_check=True,
                )
        rec = nrm.tile([P, IC], F32, tag="rec")
        nc.vector.reciprocal(rec, den)
        ocat = nrm.tile([P, IC], F32, tag="ocat")
        nc.vector.tensor_mul(out=ocat, in0=av, in1=rec)
        for m in range(IC // P):
            pp = cpsum.tile([P, 512], F32, tag="cps", name="pp")[:, :P]
            nc.tensor.matmul(pp, ocat[:, P * m:P * (m + 1)], ow,
                             start=True, stop=True)
            nc.vector.tensor_add(
                out=ostage[:, ic * (IC // P) + m, :], in0=pp, in1=ob)

    nc.sync.dma_start(out=d_out.rearrange("(o p) c -> p o c", p=P), in_=ostage)
    ctx.close()


# --------------------------------------------------------------------------
# Host side: input prep, sharding, run, gather.
# --------------------------------------------------------------------------

def _fold_qk(dwk, dwb, g, b, mu, var, pwk, pwb):
    A = g / np.sqrt(var + BN_EPS)
    dw = dwk.reshape(9, C)
    Wt = (dw * A[None, :])[:, :, None] * pwk[None, :, :]       # [9, c, f]
    const = ((A * (dwb - mu) + b)[None, :] @ pwk)[0] + pwb     # [f]
    return np.ascontiguousarray(Wt.transpose(1, 0, 2)).astype(np.float32), \
        const.astype(np.float32).reshape(C, 1)


def make_core_inputs(inputs, core):
    b, rb = core // 4, core % 4
    x = np.asarray(inputs['x'], np.float32)
    xT = x[b].T.reshape(C, L, W)
    xpad = np.zeros((C, L + 2, W + 2), np.float32)
    xpad[:, 1:65, 1:65] = xT
    xpad_q = np.ascontiguousarray(xpad[:, 16 * rb:16 * rb + 18, :])

    wq, cq = _fold_qk(*(np.asarray(inputs['q_' + k], np.float32)
                        for k in ('dwk', 'dwb', 'g', 'b', 'mu', 'var', 'pwk', 'pwb')))
    wk, ck = _fold_qk(*(np.asarray(inputs['k_' + k], np.float32)
                        for k in ('dwk', 'dwb', 'g', 'b', 'mu', 'var', 'pwk', 'pwb')))

    Av = np.asarray(inputs['v_g'], np.float32) / np.sqrt(
        np.asarray(inputs['v_var'], np.float32) + BN_EPS)
    dwv = np.asarray(inputs['v_dwk'], np.float32).reshape(9, C) * Av[None, :]  # [9, C]
    wvdiag = np.zeros((C, 9, C), np.float32)
    idx = np.arange(C)
    for t in range(9):
        wvdiag[idx, t, idx] = dwv[t]
    pwk_v = np.asarray(inputs['v_pwk'], np.float32)
    cv = ((Av * (np.asarray(inputs['v_dwb'], np.float32)
                 - np.asarray(inputs['v_mu'], np.float32))
           + np.asarray(inputs['v_b'], np.float32))[None, :] @ pwk_v)[0] \
        + np.asarray(inputs['v_pwb'], np.float32)
    out_w = np.asarray(inputs['out_w'], np.float32)
    out_b2 = np.asarray(inputs['out_b'], np.float32) + cv @ out_w
    ob_rep = np.broadcast_to(out_b2[None, :], (C, C))

    return {
        'xpad': np.ascontiguousarray(xpad.reshape(C, -1)),
        'xpad_q': np.ascontiguousarray(xpad_q.reshape(C, -1)),
        'wq': wq, 'wk': wk,
        'wvdiag': wvdiag,
        'pwkv': np.ascontiguousarray(pwk_v),
        'cq': cq, 'ck': ck,
        'out_w': np.ascontiguousarray(out_w),
        'ob_rep': np.ascontiguousarray(ob_rep),
    }


_CACHE = {}


def get_program():
    if 'nc' not in _CACHE:
        _CACHE['nc'] = build_program()
    return _CACHE['nc']


def run_cores(in_maps, trace=False, **kw):
    nc = get_program()
    return bass_utils.run_bass_kernel_spmd(
        nc, in_maps, core_ids=list(range(8)), trace=trace, **kw)


def kernel(**inputs):
    in_maps = [make_core_inputs(inputs, core) for core in range(8)]
    res = run_cores(in_maps, trace=False)
    out = np.zeros((B, N, C), np.float32)
    for core in range(8):
        b, rb = core // 4, core % 4
        out[b, rb * NB:(rb + 1) * NB] = res.results[core]['out']
    return out



# revision 6
# speedup vs baseline: 1.4176x; 1.4176x over previous
# Trainium2 Bass kernel for nn_ConvAttention (B=2, N=4096 (64x64), C=128, H=4, DH=32).
#
# Sharding: 8 cores = (batch b in {0,1}) x (query-row-block rb in {0..3}).
# Each core computes the full k/v separable convs for its batch image (bf16),
# the q conv for its 1024-row block (+halo rows, host-supplied), then
# flash-style attention over its query rows for all 4 heads, and the output
# projection for its block. Host gathers the 8 blocks.
#
# Conv fold (host): depthwise 3x3 + bias + BN + pointwise 1x1 collapse into
# 9 accumulated matmuls per branch (bf16 weights); v keeps a diagonal
# depthwise + pointwise split so its output lands transposed ([n, f]) for the
# AV matmul.
#
# Attention: scores S^T[j,(h,i)] = kT.T @ qblk (bf16, zero-padded per-head
# q so one 128-contraction matmul covers 2 heads). exp on ACT with the
# 1/sqrt(dh) scale folded in, writing fp8e4m3 probabilities. AV and the
# softmax denominator run as fp8 DoubleRow matmuls (two j-tiles packed per
# matmul via the [K,2,*] layout, 2x PE rate); the denominator uses an
# all-ones fp8 stationary operand and lands next to av in the same PSUM
# bank. Conv chunks are interleaved with attention pairs so the tensor
# engine hides conv under the ACT-bound exp stream.

import numpy as np
import ml_dtypes

import concourse.bass as bass
import concourse.tile as tile
from concourse import bacc, mybir
from concourse import bass_utils

B, L, W, C, H = 2, 64, 64, 128, 4
DH = C // H          # 32
N = L * W            # 4096
NB = N // 4          # 1024 query rows per core
SCALE = float(DH) ** -0.5
BN_EPS = 1e-3
P = 128
IC = 256             # i-chunk (query columns per inner pass)
N_IC = NB // IC      # 4
NJT = N // P         # 32 j tiles
NJP = NJT // 2       # 16 j-tile pairs

F32 = mybir.dt.float32
BF16 = mybir.dt.bfloat16
FP8 = mybir.dt.float8e4
DR = mybir.MatmulPerfMode.DoubleRow


def build_program():
    """Build and compile the SPMD single-core program (same on all 8 cores)."""
    nc = bacc.Bacc(
        "TRN2",
        target_bir_lowering=False,
        debug=False,
        enable_asserts=False,
        num_devices=8,
    )

    def din(name, shape, dt):
        return nc.dram_tensor(name, list(shape), dt, kind="ExternalInput").ap()

    d = {
        'xpad': din("xpad", (P, 66 * 66), BF16),
        'xpad_q': din("xpad_q", (P, 18 * 66), BF16),
        'wq': din("wq", (P, 9, P), BF16),
        'wk': din("wk", (P, 9, P), BF16),
        'wvdiag': din("wvdiag", (P, 9, P), BF16),
        'pwkv': din("pwkv", (P, P), BF16),
        'cq': din("cq", (P, 1), F32),
        'ck': din("ck", (P, 1), F32),
        'out_w': din("out_w", (P, P), BF16),
        'ob_rep': din("ob_rep", (P, P), F32),
    }
    d_out = nc.dram_tensor("out", [NB, C], F32, kind="ExternalOutput").ap()

    with tile.TileContext(nc) as tc:
        with nc.allow_low_precision(reason="bf16/fp8 attention pipeline"):
            _build(tc, d, d_out)

    nc.compile()
    return nc


def _build(tc, d, d_out):
    from contextlib import ExitStack
    nc = tc.nc
    Exp = mybir.ActivationFunctionType.Exp
    Add = mybir.AluOpType.add
    Mult = mybir.AluOpType.mult

    ctx = ExitStack()
    persist = ctx.enter_context(tc.tile_pool(name="persist", bufs=1))
    # PSUM: scp 2 x [128,1024] f32 = 4 banks; avp [128,4,512] f32 = 4 banks.
    scp = ctx.enter_context(tc.tile_pool(name="scp", bufs=2, space="PSUM"))
    avp = ctx.enter_context(tc.tile_pool(name="avp", bufs=1, space="PSUM"))
    ptp = ctx.enter_context(tc.tile_pool(name="ptp", bufs=8))
    nrm = ctx.enter_context(tc.tile_pool(name="nrm", bufs=2))
    ost = ctx.enter_context(tc.tile_pool(name="ost", bufs=2))

    # ---- persistent SBUF ----
    xpad = persist.tile([P, 66, 66], BF16)
    xq = persist.tile([P, 18, 66], BF16)
    wq = persist.tile([P, 9, P], BF16)
    wk = persist.tile([P, 9, P], BF16)
    wvdiag = persist.tile([P, 9, P], BF16)
    pwkv = persist.tile([P, P], BF16)
    cq = persist.tile([P, 1], F32)
    ck = persist.tile([P, 1], F32)
    ow = persist.tile([P, P], BF16)
    ob = persist.tile([P, P], F32)

    kT = persist.tile([P, N], BF16)          # [c, j] all heads stacked
    qblk = persist.tile([P, H, NB], BF16)    # per-head zero-padded q
    yvT = persist.tile([P, N], BF16)         # v depthwise output [c, n]
    # Block-diagonal masked v: vsbd[p, t, h, f] = v[t*128+p, f] if f in head h
    # else 0, so full-partition DoubleRow AV matmuls can accumulate all 4
    # heads into one PSUM region (DR forbids dst partition offsets).
    vsbd = persist.tile([P, NJT, H, P], FP8)
    onesd = persist.tile([P, 2, H, P], FP8)  # same mask with ones (denominator)

    # ---- input DMAs, ordered so k-conv can start earliest ----
    xpad_src = d['xpad'].rearrange("p (a b) -> p a b", b=66)
    nc.sync.dma_start(out=wk, in_=d['wk'])
    nc.sync.dma_start(out=xpad[:, :35], in_=xpad_src[:, :35])
    nc.sync.dma_start(out=wq, in_=d['wq'])
    nc.sync.dma_start(out=xq, in_=d['xpad_q'].rearrange("p (a b) -> p a b", b=66))
    nc.sync.dma_start(out=xpad[:, 35:], in_=xpad_src[:, 35:])
    nc.sync.dma_start(out=wvdiag, in_=d['wvdiag'])
    nc.sync.dma_start(out=pwkv, in_=d['pwkv'])
    nc.sync.dma_start(out=cq, in_=d['cq'])
    nc.sync.dma_start(out=ck, in_=d['ck'])
    nc.sync.dma_start(out=ow, in_=d['out_w'])
    nc.sync.dma_start(out=ob, in_=d['ob_rep'])

    nc.gpsimd.memset(qblk, 0.0)
    nc.gpsimd.memset(vsbd, 0.0)
    nc.gpsimd.memset(onesd, 0.0)
    for h in range(H):
        nc.gpsimd.memset(onesd[:, :, h, DH * h:DH * (h + 1)], 1.0)

    # ---- conv helpers: chunk s covers image rows [8s, 8s+8) ----
    def conv_chunk(wgt, src, s, ps):
        for t in range(9):
            dl, dw = t // 3, t % 3
            win = src[:, 8 * s + dl:8 * s + dl + 8, dw:dw + 64]
            nc.tensor.matmul(ps, wgt[:, t], win, start=(t == 0), stop=(t == 8))

    def k_chunk(s):
        ps = scp.tile([P, 1024], F32, tag="sc", name="kps")[:, :512]
        conv_chunk(wk, xpad, s, ps)
        nc.vector.tensor_scalar_add(kT[:, 512 * s:512 * (s + 1)], ps, ck)

    def v_chunk(s):
        ps = scp.tile([P, 1024], F32, tag="sc", name="vps")[:, :512]
        conv_chunk(wvdiag, xpad, s, ps)
        nc.vector.tensor_copy(yvT[:, 512 * s:512 * (s + 1)], ps)
        pw = scp.tile([P, 1024], F32, tag="sc", name="pws")[:, :512]
        pwv = pw.rearrange("p (t f) -> p t f", f=P)
        for t in range(4):
            nc.tensor.matmul(pwv[:, t], yvT[:, 512 * s + P * t:512 * s + P * (t + 1)],
                             pwkv, start=True, stop=True)
        for h in range(H):
            nc.vector.tensor_copy(
                vsbd[:, 4 * s:4 * (s + 1), h, DH * h:DH * (h + 1)],
                pwv[:, :, DH * h:DH * (h + 1)])

    # ---- q conv: 2 chunks over the 16-row halo block ----
    k_chunk(0)
    k_chunk(1)
    for s2 in range(2):
        ps = scp.tile([P, 1024], F32, tag="sc", name="qps")[:, :512]
        conv_chunk(wq, xq, s2, ps)
        for h in range(H):
            nc.vector.tensor_scalar_add(
                qblk[DH * h:DH * (h + 1), h, 512 * s2:512 * (s2 + 1)],
                ps[DH * h:DH * (h + 1), :], cq[DH * h:DH * (h + 1)])

    # ---- attention over j-tile pairs, conv interleaved ----
    av = avp.tile([P, N_IC, 512], F32)  # per ic: [:, ic, :256]=av, [:, ic, 256:]=den

    for jtp in range(NJP):
        if jtp % 2 == 0:
            s = jtp // 2
            if s + 2 < 8:
                k_chunk(s + 2)
            v_chunk(s)
        pts = []
        for ic2 in range(N_IC):
            pts.append(ptp.tile([P, 2, H, IC], FP8, tag="pt", name="pt"))
        for o in range(2):
            jt = 2 * jtp + o
            for ic2 in range(N_IC):
                sc = scp.tile([P, 1024], F32, tag="sc", name="sc")
                scv = sc.rearrange("p (h i) -> p h i", i=IC)
                for hp in range(2):
                    nc.tensor.matmul(
                        scv[:, 2 * hp:2 * hp + 2, :],
                        kT[:, P * jt:P * (jt + 1)],
                        qblk[:, 2 * hp:2 * hp + 2, IC * ic2:IC * (ic2 + 1)],
                        start=True, stop=True)
                nc.scalar.activation(out=pts[ic2][:, o], in_=sc, func=Exp,
                                     scale=SCALE)
        for ic2 in range(N_IC):
            pt = pts[ic2]
            for h in range(H):
                nc.tensor.matmul(
                    av[:, ic2, :IC],
                    vsbd[:, 2 * jtp:2 * jtp + 2, h, :],
                    pt[:, :, h, :],
                    start=(jtp == 0 and h == 0), stop=(jtp == NJP - 1 and h == H - 1),
                    perf_mode=DR, skip_group_check=True)
            for h in range(H):
                nc.tensor.matmul(
                    av[:, ic2, IC:],
                    onesd[:, :, h, :],
                    pt[:, :, h, :],
                    start=(jtp == 0 and h == 0), stop=(jtp == NJP - 1 and h == H - 1),
                    perf_mode=DR, skip_group_check=True)

    # ---- normalize + output projection, per ic ----
    for ic2 in range(N_IC):
        rec = nrm.tile([P, IC], F32, tag="rec", name="rec")
        nc.vector.reciprocal(rec, av[:, ic2, IC:])
        ocat = nrm.tile([P, IC], BF16, tag="ocat", name="ocat")
        nc.vector.tensor_tensor(out=ocat, in0=av[:, ic2, :IC], in1=rec, op=Mult)
        pp = scp.tile([P, 1024], F32, tag="sc", name="pp")
        ppv = pp.rearrange("p (m f) -> p m f", f=P)
        for m in range(2):
            nc.tensor.matmul(ppv[:, m], ocat[:, P * m:P * (m + 1)], ow,
                             start=True, stop=True)
        ostg = ost.tile([P, 2, P], F32, tag="ostg", name="ostg")
        for m in range(2):
            nc.vector.tensor_tensor(out=ostg[:, m], in0=ppv[:, m], in1=ob, op=Add)
        nc.sync.dma_start(
            out=d_out[IC * ic2:IC * (ic2 + 1)].rearrange("(m p) c -> p m c", p=P),
            in_=ostg)
    ctx.close()


# --------------------------------------------------------------------------
# Host side: input prep, sharding, run, gather.
# --------------------------------------------------------------------------

def _fold_qk(dwk, dwb, g, b, mu, var, pwk, pwb):
    A = g / np.sqrt(var + BN_EPS)
    dw = dwk.reshape(9, C)
    Wt = (dw * A[None, :])[:, :, None] * pwk[None, :, :]       # [9, c, f]
    const = ((A * (dwb - mu) + b)[None, :] @ pwk)[0] + pwb     # [f]
    return np.ascontiguousarray(Wt.transpose(1, 0, 2)), \
        const.astype(np.float32).reshape(C, 1)


def _bf16(x):
    return np.ascontiguousarray(x).astype(ml_dtypes.bfloat16)


def make_core_inputs(inputs, core):
    b, rb = core // 4, core % 4
    x = np.asarray(inputs['x'], np.float32)
    xT = x[b].T.reshape(C, L, W)
    xpad = np.zeros((C, L + 2, W + 2), np.float32)
    xpad[:, 1:65, 1:65] = xT
    xpad_q = np.ascontiguousarray(xpad[:, 16 * rb:16 * rb + 18, :])

    wq, cq = _fold_qk(*(np.asarray(inputs['q_' + k], np.float32)
                        for k in ('dwk', 'dwb', 'g', 'b', 'mu', 'var', 'pwk', 'pwb')))
    wk, ck = _fold_qk(*(np.asarray(inputs['k_' + k], np.float32)
                        for k in ('dwk', 'dwb', 'g', 'b', 'mu', 'var', 'pwk', 'pwb')))

    Av = np.asarray(inputs['v_g'], np.float32) / np.sqrt(
        np.asarray(inputs['v_var'], np.float32) + BN_EPS)
    dwv = np.asarray(inputs['v_dwk'], np.float32).reshape(9, C) * Av[None, :]  # [9, C]
    wvdiag = np.zeros((C, 9, C), np.float32)
    idx = np.arange(C)
    for t in range(9):
        wvdiag[idx, t, idx] = dwv[t]
    pwk_v = np.asarray(inputs['v_pwk'], np.float32)
    cv = ((Av * (np.asarray(inputs['v_dwb'], np.float32)
                 - np.asarray(inputs['v_mu'], np.float32))
           + np.asarray(inputs['v_b'], np.float32))[None, :] @ pwk_v)[0] \
        + np.asarray(inputs['v_pwb'], np.float32)
    out_w = np.asarray(inputs['out_w'], np.float32)
    out_b2 = np.asarray(inputs['out_b'], np.float32) + cv @ out_w
    ob_rep = np.ascontiguousarray(
        np.broadcast_to(out_b2[None, :], (C, C)).astype(np.float32))

    return {
        'xpad': _bf16(xpad.reshape(C, -1)),
        'xpad_q': _bf16(xpad_q.reshape(C, -1)),
        'wq': _bf16(wq), 'wk': _bf16(wk),
        'wvdiag': _bf16(wvdiag),
        'pwkv': _bf16(pwk_v),
        'cq': cq, 'ck': ck,
        'out_w': _bf16(out_w),
        'ob_rep': ob_rep,
    }


_CACHE = {}


def get_program():
    if 'nc' not in _CACHE:
        _CACHE['nc'] = build_program()
    return _CACHE['nc']


def run_cores(in_maps, trace=False, **kw):
    nc = get_program()
    return bass_utils.run_bass_kernel_spmd(
        nc, in_maps, core_ids=list(range(8)), trace=trace, **kw)


def kernel(**inputs):
    in_maps = [make_core_inputs(inputs, core) for core in range(8)]
    res = run_cores(in_maps, trace=False)
    out = np.zeros((B, N, C), np.float32)
    for core in range(8):
        b, rb = core // 4, core % 4
        out[b, rb * NB:(rb + 1) * NB] = res.results[core]['out']
    return out
